# revision 41
# baseline (speedup 1.0000x reference)
"""ChebConv layer (B=128, N=512, F=32, K=3) on 8 TRN2 NeuronCores.

Math: with lambda_max = 2.0, Lhat = -Ahat, Ahat = S A S with S = diag(dinv).
Folding the recursion (T0=x, T1=-Ahat x, T2=2 Ahat^2 x - x):
    u  = A q,   q  = dinv*x          (T1 = -dinv*u)
    v  = A y1,  y1 = dinv^2*u        (Ahat^2 x = dinv*v)
    out = relu( x(W0-W2) + (dinv*u)(-W1) + (dinv*v)(2 W2) + b ) + x

Sharding: data-parallel over batch, 16 samples/core as 4 groups of 4.
Host precomputes dinv exactly in f32 and prepares all layouts; adj ships
as fp8_e4m3 (4.2 MB/core vs 16.8 MB f32) - the conv terms are ~3% of the
output magnitude, so fp8 error in the A-matmuls is negligible.

Per group of 4 samples (quadrant q = partition group 32q:32q+32):
  - u-matmuls: lhsT = qn (natural layout, fp8, from host xn * 16*dinv),
    rhs = A^T chunks (fp8), out col-group q -> 4 samples stream the PE
    concurrently on 4 column groups.
  - y1T = (16*uT*dinv)*dinv on DVE (batched [128,512]), PE-transposes of
    4 [128,128] chunks give natural-layout y1n for all 4 samples at once.
  - v-matmuls like u; epilogue = 3 accumulating diagonal-tile matmuls
    per sample (x, u, v terms with rescale folded into host weights).
fp8 rescale: q' = 16q, y1' = 256*y1 keeps values in e4m3's normal range;
weights fold 1/16 and 1/128 back in.
"""

import os
import sys

sys.path.insert(0, "/opt/trn_rl_repo")

import numpy as np

import concourse.bass as bass
from concourse import bacc
import concourse.mybir as mybir
import concourse.tile as tile
from concourse.bass_utils import run_bass_kernel_spmd
from contextlib import ExitStack

B, N, F = 128, 512, 32
NCORES = 8
S = B // NCORES          # samples per core (16)
P = 128                  # SBUF partitions
C = N // P               # m-chunks per sample (4)
Q = 4                    # samples per group (one per quadrant)
G = S // Q               # groups per core (4)

f32 = mybir.dt.float32
bf16 = mybir.dt.bfloat16
f8 = mybir.dt.float8e4

_cache = {}


def _install_ntff_hook():
    """Provide antenv.axon_hooks (missing in this image) so trace=True works."""
    import contextlib
    import ctypes
    import types

    try:
        from antenv.axon_hooks import get_axon_ntff_profile_hook  # noqa: F401
        return
    except ImportError:
        pass
    so_path = "/opt/axon/libaxon_pjrt.so"
    if not os.path.exists(so_path):
        return
    lib = ctypes.CDLL(so_path)
    if not hasattr(lib, "axon_start_nrt_profile"):
        return
    lib.axon_start_nrt_profile.argtypes = [
        ctypes.POINTER(ctypes.c_int64), ctypes.c_size_t,
    ]
    lib.axon_start_nrt_profile.restype = ctypes.c_int64
    lib.axon_stop_nrt_profile.argtypes = [ctypes.c_char_p]
    lib.axon_stop_nrt_profile.restype = ctypes.c_int64

    @contextlib.contextmanager
    def _hook(output_dir, device_ids):
        import jax

        jax.devices()
        if device_ids:
            ids = (ctypes.c_int64 * len(device_ids))(*device_ids)
            rc = lib.axon_start_nrt_profile(ids, len(device_ids))
        else:
            rc = lib.axon_start_nrt_profile(None, 0)
        if rc != 0:
            raise RuntimeError(f"axon_start_nrt_profile rc={rc}")
        try:
            yield
        finally:
            n = lib.axon_stop_nrt_profile(str(output_dir).encode())
            print(f"profile: {n} file(s) written to {output_dir}", file=sys.stderr)

    mod = types.ModuleType("antenv.axon_hooks")
    state = {"hook": _hook}
    mod.get_axon_ntff_profile_hook = lambda: state["hook"]
    mod.set_axon_ntff_profile_hook = lambda h: state.update(hook=h)
    sys.modules["antenv.axon_hooks"] = mod


def build_nc():
    nc = bacc.Bacc()
    adj_d = nc.declare_dram_parameter("adj8", [P, G, C, Q, N], f8, isOutput=False)
    aux_d = nc.declare_dram_parameter("aux", [P, G, 2 * N + C * Q * F + C * Q],
                                      bf16, isOutput=False)
    wept_d = nc.declare_dram_parameter("wept", [P, 3, F], bf16, isOutput=False)
    id_d = nc.declare_dram_parameter("ident", [P, P], bf16, isOutput=False)
    b_d = nc.declare_dram_parameter("bcol", [P, 1], f32, isOutput=False)
    out_d = nc.declare_dram_parameter("out", [G, P, N], bf16, isOutput=True)

    dr = mybir.MatmulPerfMode.DoubleRow
    mult = mybir.AluOpType.mult

    with tile.TileContext(nc) as tc, ExitStack() as ctx:
        consts = ctx.enter_context(tc.tile_pool(name="consts", bufs=1))
        adj_pool = ctx.enter_context(tc.tile_pool(name="adj", bufs=G))
        work = ctx.enter_context(tc.tile_pool(name="work", bufs=3))
        ps_u = ctx.enter_context(tc.tile_pool(name="psu", bufs=2, space="PSUM"))
        ps_v = ctx.enter_context(tc.tile_pool(name="psv", bufs=2, space="PSUM"))
        ps_a = ctx.enter_context(tc.tile_pool(name="psa", bufs=2, space="PSUM"))
        ps_t = ctx.enter_context(tc.tile_pool(name="pst", bufs=1, space="PSUM"))
        ps_w = ctx.enter_context(tc.tile_pool(name="psw", bufs=1, space="PSUM"))

        # PE warm-up: dependency-free matmuls bridge from engine start
        # (~6us) to the first adj chunk arrival (~12us) so HAM hits K=8/8
        # right as real work begins and the warm PE can track the DMA
        # wavefront (a cold PE falls behind it and never warms).
        warm = consts.tile([P, N // 2], bf16, tag="warm")
        nc.vector.memset(warm, 0.0)
        wps = ps_w.tile([F, N // 2], f32, tag="wps")
        for _ in range(36):
            nc.tensor.matmul(wps, warm[:, 0:F], warm, start=True, stop=True)

        AUXW = 2 * N + C * Q * F + C * Q
        # Spread input DMAs over three DGE paths (SP ring, ACT ring, SWDGE):
        # each HWDGE ring tops out near ~200 GB/s. Group 0's adj arrives
        # chunk-by-chunk so compute starts early; consts ride SWDGE.
        ats = [adj_pool.tile([P, C, Q, N], f8, tag=f"at{g}", name=f"at{g}")
               for g in range(G)]
        auxs = [adj_pool.tile([P, AUXW], bf16, tag=f"aux{g}", name=f"aux{g}")
               for g in range(G)]
        # aux rides the ACT ring ahead of everything; each group's adj is
        # chunked round-robin over the SP and SWDGE queues so groups land
        # in order at aggregate HBM bandwidth (each ring alone sustains
        # only ~1/3 of it). g0's SWDGE chunks go before the consts - the
        # Q7 issues ~1 DMA/us and the consts aren't needed until ~14us.
        for g in range(G):
            nc.scalar.dma_start(out=auxs[g], in_=aux_d[:, g])
        # SWDGE queue order matters (~1us Q7 issue each): g0's first odd
        # chunk, then ident (transposes need it ~15us), then the rest.
        nc.gpsimd.dma_start(out=ats[0][:, 1], in_=adj_d[:, 0, 1])
        ident = consts.tile([P, P], bf16, tag="ident")
        nc.gpsimd.dma_start(out=ident, in_=id_d[:, :])
        nc.gpsimd.dma_start(out=ats[0][:, 3], in_=adj_d[:, 0, 3])
        wept = consts.tile([P, 3, F], bf16, tag="wept")
        nc.gpsimd.dma_start(out=wept, in_=wept_d[:, :, :])
        bcol = consts.tile([P, 1], f32, tag="bcol")
        nc.gpsimd.dma_start(out=bcol, in_=b_d[:, :])
        for g in range(G):
            for c in range(C):
                if g == 0 and c % 2 == 1:
                    continue
                eng = nc.sync if (c % 2 == 0) else nc.gpsimd
                eng.dma_start(out=ats[g][:, c], in_=adj_d[:, g, c])

        def stage_dma(g):
            aux = auxs[g]
            return {"at": ats[g],
                    "xt": aux[:, 0:N],
                    "s32": aux[:, N:2 * N],
                    "xn": aux[:, 2 * N:2 * N + C * Q * F].rearrange(
                        "p (c q f) -> p c q f", c=C, q=Q),
                    "sP": aux[:, 2 * N + C * Q * F:].rearrange(
                        "p (c q o) -> p c q o", c=C, q=Q)}

        def stage_q(st):
            """qn = xn * (16*dinv) (fp8); s232 = 16*dinv^2 off critical path."""
            qn = work.tile([P, C, Q, F], f8, tag="qn")
            for c in range(C):
                nc.vector.tensor_mul(
                    qn[:, c], st["xn"][:, c],
                    st["sP"][:, c].broadcast_to([P, Q, F]),
                )
            s232 = work.tile([P, N], bf16, tag="s232")
            nc.vector.scalar_tensor_tensor(
                out=s232, in0=st["s32"], scalar=16.0, in1=st["s32"],
                op0=mult, op1=mult,
            )
            st["qn"] = qn
            st["s232"] = s232

        def stage_u(st):
            """u' = A q' per sample; c-inner issue order keeps 4 col groups
            streaming concurrently (PE starts are strict FIFO)."""
            at, qn = st["at"], st["qn"]
            uT = ps_u.tile([P, N], f32, tag="uT")
            for c in range(C):
                for q in range(Q):
                    nc.tensor.matmul(
                        uT[32 * q:32 * q + 32, :], qn[:, c, q, :], at[:, c, q, :],
                        start=(c == 0), stop=(c == C - 1),
                        tile_position=(0, 32 * q),
                    )
            st["uT"] = uT

        def stage_m(st):
            """ub = dinv*u' (bf16), y1T = 16*dinv*ub, transpose to y1n fp8."""
            uT, s32 = st["uT"], st["s32"]
            y1T = work.tile([P, N], bf16, tag="y1T")
            nc.vector.tensor_mul(y1T, uT, st["s232"])
            ub = work.tile([P, N], bf16, tag="ub")
            nc.vector.tensor_mul(ub, uT, s32)
            ytp = ps_t.tile([P, C, P], bf16, tag="ytp")
            for c in range(C):
                nc.tensor.transpose(ytp[:, c, :], y1T[:, 128 * c:128 * (c + 1)], ident)
            y1n = work.tile([P, C, Q, F], f8, tag="y1n")
            for c in range(C):
                nc.scalar.activation(
                    out=y1n[:, c],
                    in_=ytp[:, c, :].rearrange("p (q f) -> p q f", q=Q),
                    func=mybir.ActivationFunctionType.Copy,
                )
            st["ub"] = ub
            st["y1n"] = y1n

        def stage_v(st):
            """v' = A y1' per sample, c-inner issue order."""
            at, y1n = st["at"], st["y1n"]
            vT = ps_v.tile([P, N], f32, tag="vT")
            for c in range(C):
                for q in range(Q):
                    nc.tensor.matmul(
                        vT[32 * q:32 * q + 32, :], y1n[:, c, q, :], at[:, c, q, :],
                        start=(c == 0), stop=(c == C - 1),
                        tile_position=(0, 32 * q),
                    )
            st["vT"] = vT

        def stage_e(st, g):
            """vb = dinv*v', epilogue matmuls, relu+bias, residual, DMA out."""
            vT, s32, xt, ub = st["vT"], st["s32"], st["xt"], st["ub"]
            vb = work.tile([P, N], bf16, tag="vb")
            nc.vector.tensor_mul(vb, vT, s32)
            acc = ps_a.tile([P, N], f32, tag="acc")
            for t, rhs4 in ((0, xt), (1, ub), (2, vb)):
                for q in range(Q):
                    sl = slice(32 * q, 32 * q + 32)
                    nc.tensor.matmul(acc[sl, :], wept[sl, t, :], rhs4[sl, :],
                                     start=(t == 0), stop=(t == 2),
                                     tile_position=(32 * q, 32 * q))
            r4 = work.tile([P, N], bf16, tag="r4")
            nc.scalar.activation(
                out=r4, in_=acc, func=mybir.ActivationFunctionType.Relu,
                bias=bcol, scale=1.0,
            )
            o4 = work.tile([P, N], bf16, tag="o4")
            H = N // 2
            nc.vector.tensor_add(o4[:, 0:H], r4[:, 0:H], xt[:, 0:H])
            nc.sync.dma_start(out=out_d[g][:, 0:H], in_=o4[:, 0:H])
            nc.vector.tensor_add(o4[:, H:N], r4[:, H:N], xt[:, H:N])
            nc.sync.dma_start(out=out_d[g][:, H:N], in_=o4[:, H:N])

        pipe = {}
        for g in range(G):
            pipe[g] = stage_dma(g)
        for i in range(G + 2):
            if 0 <= i - 1 < G:
                stage_m(pipe[i - 1])
                stage_v(pipe[i - 1])
            if 0 <= i - 2 < G:
                stage_e(pipe[i - 2], i - 2)
                del pipe[i - 2]
            if i < G:
                stage_q(pipe[i])
                stage_u(pipe[i])

    nc.finalize()
    return nc


def kernel(adj, x, W, b):
    import ml_dtypes

    adj = np.ascontiguousarray(adj, dtype=np.float32)
    x = np.ascontiguousarray(x, dtype=np.float32)
    W = np.asarray(W, dtype=np.float32)
    b = np.asarray(b, dtype=np.float32)

    f8np = ml_dtypes.float8_e4m3
    bfnp = ml_dtypes.bfloat16

    deg = adj.sum(-1)                                    # [B, N] exact f32
    dinv = np.where(deg > 0, 1.0 / np.sqrt(deg), 0.0).astype(np.float32)

    # epilogue weights with fp8 rescales folded in (q' = 16q, y1' = 256 y1)
    w0 = (W[0] - W[2])
    w1 = (-W[1]) / 16.0
    w2 = W[2] / 128.0
    wept = np.tile(np.stack([w0, w1, w2], axis=1), (4, 1, 1)).astype(bfnp)  # [128,3,32]
    ident = np.eye(P, dtype=np.float32).astype(bfnp)
    bcol = np.tile(b.reshape(1, F), (4, 1)).reshape(P, 1).astype(np.float32)

    if "nc" not in _cache:
        _cache["nc"] = build_nc()
    nc = _cache["nc"]

    in_maps = []
    for i in range(NCORES):
        sl = slice(i * S, (i + 1) * S)
        a = adj[sl]      # [16, 512, 512]
        xs = x[sl]       # [16, 512, 32]
        dv = dinv[sl]    # [16, 512]

        # adj8[p, g, c, q, n] = A_{4g+q}[n, 128c+p] (= A^T chunks)
        adj8 = np.ascontiguousarray(
            a.transpose(0, 2, 1).reshape(G, Q, C, P, N).transpose(3, 0, 2, 1, 4)
        ).astype(f8np)
        # aux[p, g, :] = concat(xt | s32 | xn | sP) per group
        # xt[32q+f, g, n] = x[4g+q][n, f]^T
        xt4 = (xs.transpose(0, 2, 1).reshape(G, Q, F, N).reshape(G, P, N)
               .transpose(1, 0, 2))                                   # [P, G, N]
        # s32[32q+f, g, n] = dinv[4g+q][n]
        s32 = (np.broadcast_to(dv.reshape(G, Q, 1, N), (G, Q, F, N))
               .reshape(G, P, N).transpose(1, 0, 2))                  # [P, G, N]
        # xn[p, g, c*q*f] = x[4g+q][128c+p, f]
        xn4 = (xs.reshape(G, Q, C, P, F).transpose(3, 0, 2, 1, 4)
               .reshape(P, G, C * Q * F))
        # sP[p, g, c*q] = 16*dinv[4g+q][128c+p]
        sP = ((16.0 * dv).reshape(G, Q, C, P).transpose(3, 0, 2, 1)
              .reshape(P, G, C * Q))
        aux = np.ascontiguousarray(
            np.concatenate([xt4, s32, xn4, sP], axis=2)
        ).astype(bfnp)

        in_maps.append({
            "adj8": adj8,
            "aux": aux,
            "wept": wept,
            "ident": ident,
            "bcol": bcol,
        })

    trace = os.environ.get("KERNEL_TRACE") == "1"
    kw = {}
    if trace:
        _install_ntff_hook()
        import concourse.bass_utils as _bu
        _bu.upload_artifacts = lambda t: t  # no bucket in this container
        kw["tmpdir"] = os.environ.get("KERNEL_TRACE_DIR") or None
    res = run_bass_kernel_spmd(
        nc, in_maps, core_ids=list(range(NCORES)), trace=trace, **kw,
    )
    if trace and res.exec_time_ns is not None:
        print(f"HW exec time: {res.exec_time_ns} ns")

    # out[g, 32q+o, n] -> sample 4g+q, [n, o]
    outs = []
    for i in range(NCORES):
        og = np.asarray(res.results[i]["out"]).astype(np.float32)  # [G, 128, 512]
        outs.append(og.reshape(G, Q, F, N).transpose(0, 1, 3, 2).reshape(S, N, F))
    return np.ascontiguousarray(np.concatenate(outs, axis=0))


# revision 42
# speedup vs baseline: 1.0684x; 1.0684x over previous
"""ChebConv layer (B=128, N=512, F=32, K=3) on 8 TRN2 NeuronCores.

Math: with lambda_max = 2.0, Lhat = -Ahat, Ahat = S A S with S = diag(dinv).
Folding the recursion (T0=x, T1=-Ahat x, T2=2 Ahat^2 x - x):
    u  = A q,   q  = dinv*x          (T1 = -dinv*u)
    v  = A y1,  y1 = dinv^2*u        (Ahat^2 x = dinv*v)
    out = relu( x(W0-W2) + (dinv*u)(-W1) + (dinv*v)(2 W2) + b ) + x

Sharding: data-parallel over batch, 16 samples/core as 4 groups of 4.
Host precomputes dinv exactly in f32 and prepares all layouts; adj ships
as fp8_e4m3 (4.2 MB/core vs 16.8 MB f32) - the conv terms are ~3% of the
output magnitude, so fp8 error in the A-matmuls is negligible.

Per group of 4 samples (quadrant q = partition group 32q:32q+32):
  - u-matmuls: lhsT = qn (natural layout, fp8, from host xn * 16*dinv),
    rhs = A^T chunks (fp8), out col-group q -> 4 samples stream the PE
    concurrently on 4 column groups.
  - y1T = (16*uT*dinv)*dinv on DVE (batched [128,512]), PE-transposes of
    4 [128,128] chunks give natural-layout y1n for all 4 samples at once.
  - v-matmuls like u; epilogue = 3 accumulating diagonal-tile matmuls
    per sample (x, u, v terms with rescale folded into host weights).
fp8 rescale: q' = 16q, y1' = 256*y1 keeps values in e4m3's normal range;
weights fold 1/16 and 1/128 back in.
"""

import os
import sys

sys.path.insert(0, "/opt/trn_rl_repo")

import numpy as np

import concourse.bass as bass
from concourse import bacc
import concourse.mybir as mybir
import concourse.tile as tile
from concourse.bass_utils import run_bass_kernel_spmd
from contextlib import ExitStack

B, N, F = 128, 512, 32
NCORES = 8
S = B // NCORES          # samples per core (16)
P = 128                  # SBUF partitions
C = N // P               # m-chunks per sample (4)
Q = 4                    # samples per group (one per quadrant)
G = S // Q               # groups per core (4)

f32 = mybir.dt.float32
bf16 = mybir.dt.bfloat16
f8 = mybir.dt.float8e4

_cache = {}


def _install_ntff_hook():
    """Provide antenv.axon_hooks (missing in this image) so trace=True works."""
    import contextlib
    import ctypes
    import types

    try:
        from antenv.axon_hooks import get_axon_ntff_profile_hook  # noqa: F401
        return
    except ImportError:
        pass
    so_path = "/opt/axon/libaxon_pjrt.so"
    if not os.path.exists(so_path):
        return
    lib = ctypes.CDLL(so_path)
    if not hasattr(lib, "axon_start_nrt_profile"):
        return
    lib.axon_start_nrt_profile.argtypes = [
        ctypes.POINTER(ctypes.c_int64), ctypes.c_size_t,
    ]
    lib.axon_start_nrt_profile.restype = ctypes.c_int64
    lib.axon_stop_nrt_profile.argtypes = [ctypes.c_char_p]
    lib.axon_stop_nrt_profile.restype = ctypes.c_int64

    @contextlib.contextmanager
    def _hook(output_dir, device_ids):
        import jax

        jax.devices()
        if device_ids:
            ids = (ctypes.c_int64 * len(device_ids))(*device_ids)
            rc = lib.axon_start_nrt_profile(ids, len(device_ids))
        else:
            rc = lib.axon_start_nrt_profile(None, 0)
        if rc != 0:
            raise RuntimeError(f"axon_start_nrt_profile rc={rc}")
        try:
            yield
        finally:
            n = lib.axon_stop_nrt_profile(str(output_dir).encode())
            print(f"profile: {n} file(s) written to {output_dir}", file=sys.stderr)

    mod = types.ModuleType("antenv.axon_hooks")
    state = {"hook": _hook}
    mod.get_axon_ntff_profile_hook = lambda: state["hook"]
    mod.set_axon_ntff_profile_hook = lambda h: state.update(hook=h)
    sys.modules["antenv.axon_hooks"] = mod


def build_nc():
    nc = bacc.Bacc()
    adj_d = nc.declare_dram_parameter("adj8", [P, G, C, Q, N], f8, isOutput=False)
    aux_d = nc.declare_dram_parameter("aux", [P, G, 2 * N + C * Q * F + C * Q],
                                      bf16, isOutput=False)
    wept_d = nc.declare_dram_parameter("wept", [P, 3, F], bf16, isOutput=False)
    id_d = nc.declare_dram_parameter("ident", [P, P], bf16, isOutput=False)
    b_d = nc.declare_dram_parameter("bcol", [P, 1], f32, isOutput=False)
    out_d = nc.declare_dram_parameter("out", [G, P, N], bf16, isOutput=True)

    dr = mybir.MatmulPerfMode.DoubleRow
    mult = mybir.AluOpType.mult

    with tile.TileContext(nc) as tc, ExitStack() as ctx:
        consts = ctx.enter_context(tc.tile_pool(name="consts", bufs=1))
        adj_pool = ctx.enter_context(tc.tile_pool(name="adj", bufs=G))
        work = ctx.enter_context(tc.tile_pool(name="work", bufs=3))
        ps_u = ctx.enter_context(tc.tile_pool(name="psu", bufs=2, space="PSUM"))
        ps_v = ctx.enter_context(tc.tile_pool(name="psv", bufs=2, space="PSUM"))
        ps_a = ctx.enter_context(tc.tile_pool(name="psa", bufs=2, space="PSUM"))
        ps_t = ctx.enter_context(tc.tile_pool(name="pst", bufs=1, space="PSUM"))
        ps_w = ctx.enter_context(tc.tile_pool(name="psw", bufs=1, space="PSUM"))

        # PE warm-up: dependency-free matmuls bridge from engine start
        # (~6us) to the first adj chunk arrival (~12us) so HAM hits K=8/8
        # right as real work begins and the warm PE can track the DMA
        # wavefront (a cold PE falls behind it and never warms).
        warm = consts.tile([P, N // 2], bf16, tag="warm")
        nc.vector.memset(warm, 0.0)
        wps = ps_w.tile([F, N // 2], f32, tag="wps")
        for _ in range(36):
            nc.tensor.matmul(wps, warm[:, 0:F], warm, start=True, stop=True)

        AUXW = 2 * N + C * Q * F + C * Q
        # Spread input DMAs over three DGE paths (SP ring, ACT ring, SWDGE):
        # each HWDGE ring tops out near ~200 GB/s. Group 0's adj arrives
        # chunk-by-chunk so compute starts early; consts ride SWDGE.
        ats = [adj_pool.tile([P, C, Q, N], f8, tag=f"at{g}", name=f"at{g}")
               for g in range(G)]
        auxs = [adj_pool.tile([P, AUXW], bf16, tag=f"aux{g}", name=f"aux{g}")
               for g in range(G)]
        # aux rides the ACT ring ahead of everything; each group's adj is
        # chunked round-robin over the SP and SWDGE queues so groups land
        # in order at aggregate HBM bandwidth (each ring alone sustains
        # only ~1/3 of it). g0's SWDGE chunks go before the consts - the
        # Q7 issues ~1 DMA/us and the consts aren't needed until ~14us.
        for g in range(G):
            nc.scalar.dma_start(out=auxs[g], in_=aux_d[:, g])
        # SWDGE queue order matters (~1us Q7 issue each): g0's first odd
        # chunk, then ident (transposes need it ~15us), then the rest.
        nc.gpsimd.dma_start(out=ats[0][:, 1], in_=adj_d[:, 0, 1])
        ident = consts.tile([P, P], bf16, tag="ident")
        nc.gpsimd.dma_start(out=ident, in_=id_d[:, :])
        nc.gpsimd.dma_start(out=ats[0][:, 3], in_=adj_d[:, 0, 3])
        wept = consts.tile([P, 3, F], bf16, tag="wept")
        nc.gpsimd.dma_start(out=wept, in_=wept_d[:, :, :])
        bcol = consts.tile([P, 1], f32, tag="bcol")
        nc.gpsimd.dma_start(out=bcol, in_=b_d[:, :])
        for g in range(G):
            for c in range(C):
                if g == 0 and c % 2 == 1:
                    continue
                eng = nc.sync if (c % 2 == 0) else nc.gpsimd
                eng.dma_start(out=ats[g][:, c], in_=adj_d[:, g, c])

        def stage_dma(g):
            aux = auxs[g]
            return {"at": ats[g],
                    "xt": aux[:, 0:N],
                    "s32": aux[:, N:2 * N],
                    "xn": aux[:, 2 * N:2 * N + C * Q * F].rearrange(
                        "p (c q f) -> p c q f", c=C, q=Q),
                    "sP": aux[:, 2 * N + C * Q * F:].rearrange(
                        "p (c q o) -> p c q o", c=C, q=Q)}

        def stage_q(st):
            """qn = xn * (16*dinv) (fp8); s232 = 16*dinv^2 off critical path."""
            qn = work.tile([P, C, Q, F], f8, tag="qn")
            for c in range(C):
                nc.vector.tensor_mul(
                    qn[:, c], st["xn"][:, c],
                    st["sP"][:, c].broadcast_to([P, Q, F]),
                )
            s232 = work.tile([P, N], bf16, tag="s232")
            nc.vector.scalar_tensor_tensor(
                out=s232, in0=st["s32"], scalar=16.0, in1=st["s32"],
                op0=mult, op1=mult,
            )
            st["qn"] = qn
            st["s232"] = s232

        def stage_u(st):
            """u' = A q' per sample; c-inner issue order keeps 4 col groups
            streaming concurrently (PE starts are strict FIFO)."""
            at, qn = st["at"], st["qn"]
            uT = ps_u.tile([P, N], f32, tag="uT")
            for c in range(C):
                for q in range(Q):
                    nc.tensor.matmul(
                        uT[32 * q:32 * q + 32, :], qn[:, c, q, :], at[:, c, q, :],
                        start=(c == 0), stop=(c == C - 1),
                        tile_position=(0, 32 * q),
                    )
            st["uT"] = uT

        def stage_m(st):
            """ub = dinv*u' (bf16), y1T = 16*dinv*ub, transpose to y1n fp8."""
            uT, s32 = st["uT"], st["s32"]
            y1T = work.tile([P, N], bf16, tag="y1T")
            nc.vector.tensor_mul(y1T, uT, st["s232"])
            ub = work.tile([P, N], bf16, tag="ub")
            nc.vector.tensor_mul(ub, uT, s32)
            ytp = ps_t.tile([P, C, P], bf16, tag="ytp")
            for c in range(C):
                nc.tensor.transpose(ytp[:, c, :], y1T[:, 128 * c:128 * (c + 1)], ident)
            y1n = work.tile([P, C, Q, F], f8, tag="y1n")
            for c in range(C):
                nc.scalar.activation(
                    out=y1n[:, c],
                    in_=ytp[:, c, :].rearrange("p (q f) -> p q f", q=Q),
                    func=mybir.ActivationFunctionType.Copy,
                )
            st["ub"] = ub
            st["y1n"] = y1n

        def stage_v(st):
            """v' = A y1' per sample, c-inner issue order."""
            at, y1n = st["at"], st["y1n"]
            vT = ps_v.tile([P, N], f32, tag="vT")
            for c in range(C):
                for q in range(Q):
                    nc.tensor.matmul(
                        vT[32 * q:32 * q + 32, :], y1n[:, c, q, :], at[:, c, q, :],
                        start=(c == 0), stop=(c == C - 1),
                        tile_position=(0, 32 * q),
                    )
            st["vT"] = vT

        def stage_e(st, g):
            """vb = dinv*v', epilogue matmuls, relu+bias, residual, DMA out."""
            vT, s32, xt, ub = st["vT"], st["s32"], st["xt"], st["ub"]
            vb = work.tile([P, N], bf16, tag="vb")
            nc.vector.tensor_mul(vb, vT, s32)
            acc = ps_a.tile([P, N], f32, tag="acc")
            for t, rhs4 in ((0, xt), (1, ub), (2, vb)):
                for q in range(Q):
                    sl = slice(32 * q, 32 * q + 32)
                    nc.tensor.matmul(acc[sl, :], wept[sl, t, :], rhs4[sl, :],
                                     start=(t == 0), stop=(t == 2),
                                     tile_position=(32 * q, 32 * q))
            r4 = work.tile([P, N], bf16, tag="r4")
            nc.scalar.activation(
                out=r4, in_=acc, func=mybir.ActivationFunctionType.Relu,
                bias=bcol, scale=1.0,
            )
            o4 = work.tile([P, N], bf16, tag="o4")
            nc.vector.tensor_add(o4, r4, xt)
            nc.sync.dma_start(out=out_d[g], in_=o4)

        pipe = {}
        for g in range(G):
            pipe[g] = stage_dma(g)
        for i in range(G + 2):
            if 0 <= i - 1 < G:
                stage_m(pipe[i - 1])
                stage_v(pipe[i - 1])
            if 0 <= i - 2 < G:
                stage_e(pipe[i - 2], i - 2)
                del pipe[i - 2]
            if i < G:
                stage_q(pipe[i])
                stage_u(pipe[i])

    nc.finalize()
    return nc


def kernel(adj, x, W, b):
    import ml_dtypes

    adj = np.ascontiguousarray(adj, dtype=np.float32)
    x = np.ascontiguousarray(x, dtype=np.float32)
    W = np.asarray(W, dtype=np.float32)
    b = np.asarray(b, dtype=np.float32)

    f8np = ml_dtypes.float8_e4m3
    bfnp = ml_dtypes.bfloat16

    deg = adj.sum(-1)                                    # [B, N] exact f32
    dinv = np.where(deg > 0, 1.0 / np.sqrt(deg), 0.0).astype(np.float32)

    # epilogue weights with fp8 rescales folded in (q' = 16q, y1' = 256 y1)
    w0 = (W[0] - W[2])
    w1 = (-W[1]) / 16.0
    w2 = W[2] / 128.0
    wept = np.tile(np.stack([w0, w1, w2], axis=1), (4, 1, 1)).astype(bfnp)  # [128,3,32]
    ident = np.eye(P, dtype=np.float32).astype(bfnp)
    bcol = np.tile(b.reshape(1, F), (4, 1)).reshape(P, 1).astype(np.float32)

    if "nc" not in _cache:
        _cache["nc"] = build_nc()
    nc = _cache["nc"]

    in_maps = []
    for i in range(NCORES):
        sl = slice(i * S, (i + 1) * S)
        a = adj[sl]      # [16, 512, 512]
        xs = x[sl]       # [16, 512, 32]
        dv = dinv[sl]    # [16, 512]

        # adj8[p, g, c, q, n] = A_{4g+q}[n, 128c+p] (= A^T chunks)
        adj8 = np.ascontiguousarray(
            a.transpose(0, 2, 1).reshape(G, Q, C, P, N).transpose(3, 0, 2, 1, 4)
        ).astype(f8np)
        # aux[p, g, :] = concat(xt | s32 | xn | sP) per group
        # xt[32q+f, g, n] = x[4g+q][n, f]^T
        xt4 = (xs.transpose(0, 2, 1).reshape(G, Q, F, N).reshape(G, P, N)
               .transpose(1, 0, 2))                                   # [P, G, N]
        # s32[32q+f, g, n] = dinv[4g+q][n]
        s32 = (np.broadcast_to(dv.reshape(G, Q, 1, N), (G, Q, F, N))
               .reshape(G, P, N).transpose(1, 0, 2))                  # [P, G, N]
        # xn[p, g, c*q*f] = x[4g+q][128c+p, f]
        xn4 = (xs.reshape(G, Q, C, P, F).transpose(3, 0, 2, 1, 4)
               .reshape(P, G, C * Q * F))
        # sP[p, g, c*q] = 16*dinv[4g+q][128c+p]
        sP = ((16.0 * dv).reshape(G, Q, C, P).transpose(3, 0, 2, 1)
              .reshape(P, G, C * Q))
        aux = np.ascontiguousarray(
            np.concatenate([xt4, s32, xn4, sP], axis=2)
        ).astype(bfnp)

        in_maps.append({
            "adj8": adj8,
            "aux": aux,
            "wept": wept,
            "ident": ident,
            "bcol": bcol,
        })

    trace = os.environ.get("KERNEL_TRACE") == "1"
    kw = {}
    if trace:
        _install_ntff_hook()
        import concourse.bass_utils as _bu
        _bu.upload_artifacts = lambda t: t  # no bucket in this container
        kw["tmpdir"] = os.environ.get("KERNEL_TRACE_DIR") or None
    res = run_bass_kernel_spmd(
        nc, in_maps, core_ids=list(range(NCORES)), trace=trace, **kw,
    )
    if trace and res.exec_time_ns is not None:
        print(f"HW exec time: {res.exec_time_ns} ns")

    # out[g, 32q+o, n] -> sample 4g+q, [n, o]
    outs = []
    for i in range(NCORES):
        og = np.asarray(res.results[i]["out"]).astype(np.float32)  # [G, 128, 512]
        outs.append(og.reshape(G, Q, F, N).transpose(0, 1, 3, 2).reshape(S, N, F))
    return np.ascontiguousarray(np.concatenate(outs, axis=0))
